# revision 23
# baseline (speedup 1.0000x reference)
"""Multi-head self-attention (B=2, L=2048, H=16, dh=64) on 8 TRN2 NeuronCores.

Strategy (v8):
  - One SPMD launch; each core runs one head-pair (2 heads) of EVERY batch,
    as straight-line sections with per-batch loop bounds (padded to 128).
  - All X/W shipped bf16, host-packed [128, KC*L] chunk-major; DMAed in
    512-column pieces ordered so compute can chase the DMA stream
    (the input stream, not compute, bounds the start of the big batch):
    per section [xq piece0, (xk,xv) piece pairs, remaining xq pieces].
  - A short warm-up matmul chain on memset data ramps the PE p-state to
    full clock before real work arrives.
  - qT/kT = W.T @ X.T projections -> PSUM -> bf16 SBUF; V projected in
    [key, dh] orientation; k-proj and v-proj interleaved per arriving
    DMA piece (the engines execute their static streams IN ORDER, so a
    stalled producer ahead in the stream blocks everything behind it).
  - S^T[k, q] per head via paired K=64 matmuls (tile_position packing),
    each head's 512-wide slice in its own PSUM bank.
  - exp on ScalarE straight from PSUM; 1/sqrt(dh) folded into the
    activation scale; additive key mask only for the final (partial) key
    chunk; bf16 output.
  - A@V with the exp tile stationary and ones-augmented V moving (N=65):
    accumulates directly as [query, head*65+d] with the softmax
    denominator in column 64 -- no transposes.  PSUM `start` zeroes a
    whole bank, so each accumulator bank gets exactly one start.  The
    A@V for key chunk kc is emitted after S/exp of kc+1 (software
    pipelining).  Host performs the final divide and query-length crop.
    Output DMAs ride the GpSimd SWDGE queue, as bf16.
"""

import math
from contextlib import ExitStack

import ml_dtypes
import numpy as np

import concourse.mybir as mybir
import concourse.tile as tile
from concourse import bacc
from concourse.bass_utils import run_bass_kernel_spmd

F32 = mybir.dt.float32
BF16 = mybir.dt.bfloat16
EXP = mybir.ActivationFunctionType.Exp
NEG_BIG = 1e12

D_MODEL = 1024
L_FULL = 2048
DH = 64
N_CORES = 8
KC = D_MODEL // 128    # contraction chunks
HW = 128               # one head-pair (2 heads) per core

_nc_cache: dict = {}
TRACE = False


def _pad128(n: int) -> int:
    return min(L_FULL, max(128, int(math.ceil(n / 128)) * 128))


def _cfgs_for(ql, vl):
    """Section configs, smallest DMA footprint first."""
    B = len(ql)
    order = sorted(range(B), key=lambda b: _pad128(int(ql[b])) + 2 * _pad128(int(vl[b])))
    return tuple((_pad128(int(ql[b])), _pad128(int(vl[b]))) for b in order), order


def _pieces(L, w=512):
    """w-wide column pieces (merge a <256 tail so DMA elems stay >=512B)."""
    ps = [(o, min(w, L - o)) for o in range(0, L, w)]
    if len(ps) > 1 and ps[-1][1] < 256:
        (o, pw), (_, wt) = ps[-2], ps[-1]
        ps[-2:] = [(o, pw + wt)]
    return ps


def _build(cfgs: tuple):
    """cfgs: tuple of (LQ, LK) per batch section."""
    if cfgs in _nc_cache:
        return _nc_cache[cfgs]

    nc = bacc.Bacc("TRN2", target_bir_lowering=False, debug=False,
                   num_devices=N_CORES)
    nsec = len(cfgs)

    w_d = nc.dram_tensor("w", [128, 3 * KC * HW], BF16, kind="ExternalInput")
    kb_d = nc.dram_tensor("kb", [128, nsec], F32, kind="ExternalInput")
    secs = []
    for i, (LQ, LK) in enumerate(cfgs):
        d = dict(LQ=LQ, LK=LK, NKC=LK // 128)
        d["qtiles"] = [(o, min(512, LQ - o)) for o in range(0, LQ, 512)]
        d["xq_d"] = nc.dram_tensor(f"xq{i}", [128, KC * LQ], BF16, kind="ExternalInput")
        d["xk_d"] = nc.dram_tensor(f"xk{i}", [128, KC * LK], BF16, kind="ExternalInput")
        d["xv_d"] = nc.dram_tensor(f"xv{i}", [128, KC * LK], BF16, kind="ExternalInput")
        NP = (LQ // 128) // 2
        d["NP"] = NP
        d["out2_d"] = nc.dram_tensor(f"out2{i}", [NP * 128, 260], BF16,
                                     kind="ExternalOutput")
        if (LQ // 128) % 2:
            d["out1_d"] = nc.dram_tensor(f"out1{i}", [128, 130], BF16,
                                         kind="ExternalOutput")
        secs.append(d)

    with ExitStack() as ctx:
        tc = ctx.enter_context(tile.TileContext(nc))
        const = ctx.enter_context(tc.tile_pool(name="const", bufs=1))
        xpool = ctx.enter_context(tc.tile_pool(name="xp", bufs=1))
        qkpool = ctx.enter_context(tc.tile_pool(name="qk", bufs=1))
        vpool = ctx.enter_context(tc.tile_pool(name="vp", bufs=1))
        epool = ctx.enter_context(tc.tile_pool(name="ep", bufs=6))
        stpool = ctx.enter_context(tc.tile_pool(name="st", bufs=4))
        # PSUM (8 banks): 2 x 2-bank score tiles + 1+1 banks of O
        # accumulators + 2 x 1-bank projection slots.
        spool = ctx.enter_context(tc.tile_pool(name="ps_s", bufs=2, space="PSUM"))
        apA = ctx.enter_context(tc.tile_pool(name="ps_a", bufs=1, space="PSUM"))
        apB = ctx.enter_context(tc.tile_pool(name="ps_b", bufs=1, space="PSUM"))
        pjp = ctx.enter_context(tc.tile_pool(name="ps_pj", bufs=2, space="PSUM"))

        # ---- PE p-state warm-up: ~3.5us of throwaway matmuls on memset
        # data so the real stream starts at full clock.
        wu = const.tile([128, 512], BF16, name="wu")
        nc.vector.memset(wu, 0.0)
        wup = apA.tile([128, 390], F32, tag="avA", name="wup")
        for r in range(8):
            nc.tensor.matmul(wup, lhsT=wu[:, 0:128], rhs=wu[:, 0:390],
                             start=True, stop=True)

        # ---- input DMAs, arrival order = consumption order ----
        wt = const.tile([128, 3 * KC * HW], BF16, name="w")
        nc.sync.dma_start(out=wt[:, 0:KC * HW], in_=w_d[:, 0:KC * HW])
        kb = const.tile([128, nsec], F32, name="kb")
        nc.sync.dma_start(out=kb, in_=kb_d[:, :])
        NW = KC * HW
        first = True
        for i, d in enumerate(secs):
            LQ, LK = d["LQ"], d["LK"]
            for key, L in (("xq", LQ), ("xk", LK), ("xv", LK)):
                t = xpool.tile([128, KC * L], BF16, tag=f"{key}{i}", name=f"{key}{i}")
                d[key] = t
                d[key + "3"] = t.rearrange("p (k l) -> p k l", l=L)
                d[key + "_d3"] = d[key + "_d"][:, :].rearrange("p (k l) -> p k l", l=L)

            def xdma(key, off, w_):
                nc.sync.dma_start(out=d[key + "3"][:, :, off:off + w_],
                                  in_=d[key + "_d3"][:, :, off:off + w_])

            qp = _pieces(LQ)
            xdma("xq", *qp[0])
            if first:
                # W for the K projection rides after the first xq piece
                nc.sync.dma_start(out=wt[:, NW:2 * NW], in_=w_d[:, NW:2 * NW])
            if LK // 128 > 8:
                # multi-pass section: both qtiles run per pass, so all qT
                # pieces must precede the k/v stream
                for off, w_ in qp[1:]:
                    xdma("xq", off, w_)
            for j, (off, w_) in enumerate(_pieces(LK)):
                xdma("xk", off, w_)
                if first and j == 0:
                    # W for the V projection after the first xk piece
                    nc.sync.dma_start(out=wt[:, 2 * NW:], in_=w_d[:, 2 * NW:])
                    first = False
                xdma("xv", off, w_)
            if LK // 128 <= 8:
                for off, w_ in qp[1:]:
                    xdma("xq", off, w_)

        # ---- per-section compute; later sections' projections fill PE
        # gaps while earlier sections' attention is ACT-bound ----
        for i, d in enumerate(secs):
            LQ, LK, NKC = d["LQ"], d["LK"], d["NKC"]

            qT = qkpool.tile([128, LQ], BF16, tag=f"qT{i}", name=f"qT{i}")
            kT = qkpool.tile([128, LK], BF16, tag=f"kT{i}", name=f"kT{i}")
            v = vpool.tile([128, NKC * 130], BF16, tag=f"v{i}", name=f"v{i}")
            v4 = v.rearrange("p (m h c) -> p m h c", h=2, c=65)
            nc.vector.memset(v4[:, :, :, 64:65], 1.0)

            def proj_tile(proj, xkey, L, dst, off, tw):
                pj = pjp.tile([128, 512], F32, tag="pj", name=f"pj{i}{proj}{off}")
                for kc in range(KC):
                    nc.tensor.matmul(
                        pj[:, 0:tw],
                        lhsT=wt[:, (proj * KC + kc) * HW:(proj * KC + kc + 1) * HW],
                        rhs=d[xkey][:, kc * L + off:kc * L + off + tw],
                        start=(kc == 0), stop=(kc == KC - 1),
                    )
                nc.vector.tensor_copy(dst[:, off:off + tw], pj[:, 0:tw])

            def vproj(mc):
                pv = pjp.tile([128, 128], F32, tag="pj", name=f"pv{i}{mc}")
                for kc in range(KC):
                    nc.tensor.matmul(
                        pv,
                        lhsT=d["xv"][:, kc * LK + mc * 128:kc * LK + (mc + 1) * 128],
                        rhs=wt[:, (2 * KC + kc) * HW:(2 * KC + kc + 1) * HW],
                        start=(kc == 0), stop=(kc == KC - 1),
                    )
                nc.vector.tensor_copy(
                    v4[:, mc:mc + 1, :, 0:64],
                    pv.rearrange("p (o h c) -> p o h c", o=1, c=64))

            # program order mirrors DMA arrival: qT tile0, then (kT tile,
            # its v chunks) per arriving k/v piece, then the remaining qT
            # tiles interleaved between attention qtiles.
            proj_tile(0, "xq", LQ, qT, *d["qtiles"][0])

            # Flash-style key-range passes for long-key sections: both
            # qtiles consume each arriving k/v piece (partial O + partial
            # denominators staged in SBUF, summed at the end) so ScalarE
            # stays fed while the k/v stream is still arriving.
            multi = NKC > 8
            passes = [(0, 6), (6, NKC)] if multi else [(0, NKC)]
            if multi:
                for off2, tw2 in d["qtiles"][1:]:
                    proj_tile(0, "xq", LQ, qT, off2, tw2)
            kvps = _pieces(LK)
            for pi, (kp0, kp1) in enumerate(passes):
                for off, tw in kvps:
                    if kp0 <= off // 128 < kp1:
                        proj_tile(1, "xk", LK, kT, off, tw)
                        for mc in range(off // 128, (off + tw) // 128):
                            vproj(mc)
                for nq, (qt, qw) in enumerate(d["qtiles"]):
                    if not multi and nq + 1 < len(d["qtiles"]):
                        proj_tile(0, "xq", LQ, qT, *d["qtiles"][nq + 1])
                    nqc = qw // 128
                    nA = min(nqc, 3)
                    oA = apA.tile([128, nA * 130], F32, tag="avA",
                                  name=f"oA{i}_{pi}_{nq}")
                    oB = (apB.tile([128, 130], F32, tag="avB",
                                   name=f"oB{i}_{pi}_{nq}")
                          if nqc == 4 else None)

                    def emit_av(e, kc):
                        # PSUM `start` zeroes the whole bank, so each bank
                        # gets exactly one start (its first matmul); later
                        # disjoint slices accumulate onto the cleared bank.
                        for qc in range(nqc):
                            dst = oA if qc < 3 else oB
                            pos = qc % 3
                            for h in range(2):
                                nc.tensor.matmul(
                                    dst[:, pos * 130 + h * 65:
                                        pos * 130 + (h + 1) * 65],
                                    lhsT=e[:, h * 512 + qc * 128:
                                           h * 512 + (qc + 1) * 128],
                                    rhs=v4[:, kc:kc + 1, h:h + 1, :],
                                    start=(kc == kp0 and pos == 0 and h == 0),
                                    stop=(kc == kp1 - 1 and h == 1
                                          and pos == (nA - 1 if qc < 3 else 0)),
                                    skip_group_check=True,
                                )

                    # software pipeline: emit [S + exp for kc][A@V for kc-1]
                    # so the in-order PE stream never waits on ScalarE.
                    prev = None
                    for kc in range(kp0, kp1):
                        s = spool.tile([128, 1024], F32, tag="s",
                                       name=f"s{i}_{pi}_{nq}_{kc}")
                        s3 = s.rearrange("p (h q) -> p h q", q=512)
                        for h in range(2):
                            # each head's slice in its own PSUM bank (two
                            # concurrent tile_position matmuls into one bank
                            # is fatal on HW)
                            nc.tensor.matmul(
                                s3[:, h:h + 1, 0:qw],
                                lhsT=kT[h * 64:(h + 1) * 64,
                                        kc * 128:(kc + 1) * 128],
                                rhs=qT[h * 64:(h + 1) * 64, qt:qt + qw],
                                start=True, stop=True,
                                tile_position=(h * 64, 0),
                            )
                        e = epool.tile([128, 1024], BF16, tag="e",
                                       name=f"e{i}_{pi}_{nq}_{kc}")
                        e3 = e.rearrange("p (h q) -> p h q", q=512)
                        bias = kb[:, i:i + 1] if kc == NKC - 1 else 0.0
                        nc.scalar.activation(e3[:, :, 0:qw], s3[:, :, 0:qw],
                                             EXP, bias=bias, scale=0.125)
                        if prev is not None:
                            emit_av(prev, kc - 1)
                        prev = e
                    emit_av(prev, kp1 - 1)

                    if multi and pi == 0:
                        # stage the first pass's partials in fp32 SBUF
                        stP = stpool.tile([128, nqc * 130], F32,
                                          tag=f"stP{nq}", name=f"stP{i}_{nq}")
                        nc.vector.tensor_copy(stP[:, 0:nA * 130], oA)
                        if oB is not None:
                            nc.vector.tensor_copy(stP[:, 390:520], oB)
                        d[f"stP{nq}"] = stP
                        continue
                    # final pass: combine partials, DMA out as bf16 with
                    # query-chunk PAIRS packed per DRAM row (520B
                    # descriptors).  Host unpacks + divides by denominator.
                    st = stpool.tile([128, nqc * 130], BF16, tag="stA",
                                     name=f"st{i}_{nq}")
                    if multi:
                        stP = d[f"stP{nq}"]
                        nc.vector.tensor_add(st[:, 0:nA * 130], oA,
                                             stP[:, 0:nA * 130])
                        if oB is not None:
                            nc.vector.tensor_add(st[:, 390:520], oB,
                                                 stP[:, 390:520])
                    else:
                        nc.vector.tensor_copy(st[:, 0:nA * 130], oA)
                        if oB is not None:
                            nc.vector.tensor_copy(st[:, 390:520], oB)
                    npair = nqc // 2
                    if npair:
                        P0 = qt // 256
                        nc.sync.dma_start(
                            out=d["out2_d"][P0 * 128:(P0 + npair) * 128, :]
                            .rearrange("(c p) f -> p c f", p=128),
                            in_=st.rearrange("p (c f) -> p c f",
                                             f=260)[:, 0:npair, :])
                    if nqc % 2:
                        nc.sync.dma_start(out=d["out1_d"][:, :],
                                          in_=st[:, (nqc - 1) * 130:nqc * 130])

    nc.compile()
    _nc_cache[cfgs] = nc
    return nc


def _pack_xt(x: np.ndarray, L: int) -> np.ndarray:
    """[L_full, 1024] fp32 -> [128, KC*L] bf16, chunk-major transposed."""
    xt = np.ascontiguousarray(x[:L, :].T)           # [1024, L]
    xt = xt.reshape(KC, 128, L).transpose(1, 0, 2)  # [128, KC, L]
    return np.ascontiguousarray(xt.reshape(128, KC * L)).astype(ml_dtypes.bfloat16)


def _pack_w(W: np.ndarray, g: int) -> np.ndarray:
    """[1024, 1024] -> this core's [128, KC*128] slice, chunk-major."""
    wg = W[:, g * HW:(g + 1) * HW].reshape(KC, 128, HW).transpose(1, 0, 2)
    return np.ascontiguousarray(wg.reshape(128, KC * HW))


def kernel(Q_seq, K_seq, V_seq, q_len, v_len, WQ, WK, WV):
    Q_seq = np.asarray(Q_seq, dtype=np.float32)
    K_seq = np.asarray(K_seq, dtype=np.float32)
    V_seq = np.asarray(V_seq, dtype=np.float32)
    WQ = np.asarray(WQ, dtype=np.float32)
    WK = np.asarray(WK, dtype=np.float32)
    WV = np.asarray(WV, dtype=np.float32)
    ql = np.asarray(q_len).ravel().astype(np.int64)
    vl = np.asarray(v_len).ravel().astype(np.int64)
    B = Q_seq.shape[0]

    cfgs, order = _cfgs_for(ql, vl)
    nc = _build(cfgs)

    kb = np.zeros((128, len(cfgs)), dtype=np.float32)
    shared = {}
    for sec, b in enumerate(order):
        LQ, LK = cfgs[sec]
        shared[f"xq{sec}"] = _pack_xt(Q_seq[b], LQ)
        shared[f"xk{sec}"] = _pack_xt(K_seq[b], LK)
        shared[f"xv{sec}"] = _pack_xt(V_seq[b], LK)
        j0 = LK - 128
        kb[:, sec] = np.where(np.arange(j0, LK) < vl[b], 0.0, -np.float32(NEG_BIG))

    in_maps = []
    for g in range(N_CORES):
        m = dict(shared)
        m["w"] = np.concatenate(
            [_pack_w(WQ, g), _pack_w(WK, g), _pack_w(WV, g)],
            axis=1).astype(ml_dtypes.bfloat16)
        m["kb"] = kb
        in_maps.append(m)

    res = run_bass_kernel_spmd(nc, in_maps, list(range(N_CORES)), trace=TRACE)
    kernel.last_results = [res]
    kernel.last_exec_ns = res.exec_time_ns or 0

    O = np.zeros((B, L_FULL, D_MODEL), dtype=np.float32)
    for sec, b in enumerate(order):
        LQ = cfgs[sec][0]
        n = int(ql[b])
        NP = (LQ // 128) // 2
        for g in range(N_CORES):
            full = np.empty((LQ, 130), dtype=np.float32)
            o2 = np.asarray(res.results[g][f"out2{sec}"], dtype=np.float32)
            full[:NP * 256] = o2.reshape(NP, 128, 2, 130).transpose(
                0, 2, 1, 3).reshape(NP * 256, 130)
            if (LQ // 128) % 2:
                full[NP * 256:] = np.asarray(res.results[g][f"out1{sec}"],
                                             dtype=np.float32)
            o = full.reshape(LQ, 2, 65)
            O[b, :n, g * HW:(g + 1) * HW] = (
                o[:n, :, 0:64] / o[:n, :, 64:65]).reshape(n, HW)
    return O


# revision 24
# speedup vs baseline: 1.0106x; 1.0106x over previous
"""Multi-head self-attention (B=2, L=2048, H=16, dh=64) on 8 TRN2 NeuronCores.

Strategy (v8):
  - One SPMD launch; each core runs one head-pair (2 heads) of EVERY batch,
    as straight-line sections with per-batch loop bounds (padded to 128).
  - All X/W shipped bf16, host-packed [128, KC*L] chunk-major; DMAed in
    512-column pieces ordered so compute can chase the DMA stream
    (the input stream, not compute, bounds the start of the big batch):
    per section [xq piece0, (xk,xv) piece pairs, remaining xq pieces].
  - A short warm-up matmul chain on memset data ramps the PE p-state to
    full clock before real work arrives.
  - qT/kT = W.T @ X.T projections -> PSUM -> bf16 SBUF; V projected in
    [key, dh] orientation; k-proj and v-proj interleaved per arriving
    DMA piece (the engines execute their static streams IN ORDER, so a
    stalled producer ahead in the stream blocks everything behind it).
  - S^T[k, q] per head via paired K=64 matmuls (tile_position packing),
    each head's 512-wide slice in its own PSUM bank.
  - exp on ScalarE straight from PSUM; 1/sqrt(dh) folded into the
    activation scale; additive key mask only for the final (partial) key
    chunk; bf16 output.
  - A@V with the exp tile stationary and ones-augmented V moving (N=65):
    accumulates directly as [query, head*65+d] with the softmax
    denominator in column 64 -- no transposes.  PSUM `start` zeroes a
    whole bank, so each accumulator bank gets exactly one start.  The
    A@V for key chunk kc is emitted after S/exp of kc+1 (software
    pipelining).  Host performs the final divide and query-length crop.
    Output DMAs ride the GpSimd SWDGE queue, as bf16.
"""

import math
from contextlib import ExitStack

import ml_dtypes
import numpy as np

import concourse.mybir as mybir
import concourse.tile as tile
from concourse import bacc
from concourse.bass_utils import run_bass_kernel_spmd

F32 = mybir.dt.float32
BF16 = mybir.dt.bfloat16
EXP = mybir.ActivationFunctionType.Exp
NEG_BIG = 1e12

D_MODEL = 1024
L_FULL = 2048
DH = 64
N_CORES = 8
KC = D_MODEL // 128    # contraction chunks
HW = 128               # one head-pair (2 heads) per core

_nc_cache: dict = {}
TRACE = False


def _pad128(n: int) -> int:
    return min(L_FULL, max(128, int(math.ceil(n / 128)) * 128))


def _cfgs_for(ql, vl):
    """Section configs, smallest DMA footprint first."""
    B = len(ql)
    order = sorted(range(B), key=lambda b: _pad128(int(ql[b])) + 2 * _pad128(int(vl[b])))
    return tuple((_pad128(int(ql[b])), _pad128(int(vl[b]))) for b in order), order


def _pieces(L, w=512):
    """w-wide column pieces (merge a <256 tail so DMA elems stay >=512B)."""
    ps = [(o, min(w, L - o)) for o in range(0, L, w)]
    if len(ps) > 1 and ps[-1][1] < 256:
        (o, pw), (_, wt) = ps[-2], ps[-1]
        ps[-2:] = [(o, pw + wt)]
    return ps


def _build(cfgs: tuple):
    """cfgs: tuple of (LQ, LK) per batch section."""
    if cfgs in _nc_cache:
        return _nc_cache[cfgs]

    nc = bacc.Bacc("TRN2", target_bir_lowering=False, debug=False,
                   num_devices=N_CORES)
    nsec = len(cfgs)

    w_d = nc.dram_tensor("w", [128, 3 * KC * HW], BF16, kind="ExternalInput")
    kb_d = nc.dram_tensor("kb", [128, nsec], F32, kind="ExternalInput")
    secs = []
    for i, (LQ, LK) in enumerate(cfgs):
        d = dict(LQ=LQ, LK=LK, NKC=LK // 128)
        d["qtiles"] = [(o, min(512, LQ - o)) for o in range(0, LQ, 512)]
        d["xq_d"] = nc.dram_tensor(f"xq{i}", [128, KC * LQ], BF16, kind="ExternalInput")
        d["xk_d"] = nc.dram_tensor(f"xk{i}", [128, KC * LK], BF16, kind="ExternalInput")
        d["xv_d"] = nc.dram_tensor(f"xv{i}", [128, KC * LK], BF16, kind="ExternalInput")
        NP = (LQ // 128) // 2
        d["NP"] = NP
        d["out2_d"] = nc.dram_tensor(f"out2{i}", [NP * 128, 260], BF16,
                                     kind="ExternalOutput")
        if (LQ // 128) % 2:
            d["out1_d"] = nc.dram_tensor(f"out1{i}", [128, 130], BF16,
                                         kind="ExternalOutput")
        secs.append(d)

    with ExitStack() as ctx:
        tc = ctx.enter_context(tile.TileContext(nc))
        const = ctx.enter_context(tc.tile_pool(name="const", bufs=1))
        xpool = ctx.enter_context(tc.tile_pool(name="xp", bufs=1))
        qkpool = ctx.enter_context(tc.tile_pool(name="qk", bufs=1))
        vpool = ctx.enter_context(tc.tile_pool(name="vp", bufs=1))
        epool = ctx.enter_context(tc.tile_pool(name="ep", bufs=6))
        stpool = ctx.enter_context(tc.tile_pool(name="st", bufs=4))
        # PSUM (8 banks): 2 x 2-bank score tiles + 1+1 banks of O
        # accumulators + 2 x 1-bank projection slots.
        spool = ctx.enter_context(tc.tile_pool(name="ps_s", bufs=2, space="PSUM"))
        apA = ctx.enter_context(tc.tile_pool(name="ps_a", bufs=1, space="PSUM"))
        apB = ctx.enter_context(tc.tile_pool(name="ps_b", bufs=1, space="PSUM"))
        pjp = ctx.enter_context(tc.tile_pool(name="ps_pj", bufs=2, space="PSUM"))

        # ---- PE p-state warm-up: ~3.5us of throwaway matmuls on memset
        # data so the real stream starts at full clock.
        wu = const.tile([128, 512], BF16, name="wu")
        nc.vector.memset(wu, 0.0)
        wup = apA.tile([128, 390], F32, tag="avA", name="wup")
        for r in range(8):
            nc.tensor.matmul(wup, lhsT=wu[:, 0:128], rhs=wu[:, 0:390],
                             start=True, stop=True)

        # ---- input DMAs, arrival order = consumption order ----
        wt = const.tile([128, 3 * KC * HW], BF16, name="w")
        nc.sync.dma_start(out=wt[:, 0:KC * HW], in_=w_d[:, 0:KC * HW])
        kb = const.tile([128, nsec], F32, name="kb")
        nc.sync.dma_start(out=kb, in_=kb_d[:, :])
        NW = KC * HW
        first = True
        for i, d in enumerate(secs):
            LQ, LK = d["LQ"], d["LK"]
            for key, L in (("xq", LQ), ("xk", LK), ("xv", LK)):
                t = xpool.tile([128, KC * L], BF16, tag=f"{key}{i}", name=f"{key}{i}")
                d[key] = t
                d[key + "3"] = t.rearrange("p (k l) -> p k l", l=L)
                d[key + "_d3"] = d[key + "_d"][:, :].rearrange("p (k l) -> p k l", l=L)

            def xdma(key, off, w_):
                nc.sync.dma_start(out=d[key + "3"][:, :, off:off + w_],
                                  in_=d[key + "_d3"][:, :, off:off + w_])

            qp = _pieces(LQ)
            xdma("xq", *qp[0])
            if first:
                # W for the K projection rides after the first xq piece
                nc.sync.dma_start(out=wt[:, NW:2 * NW], in_=w_d[:, NW:2 * NW])
            if LK // 128 > 8:
                # multi-pass section: both qtiles run per pass, so all qT
                # pieces must precede the k/v stream
                for off, w_ in qp[1:]:
                    xdma("xq", off, w_)
            for j, (off, w_) in enumerate(_pieces(LK)):
                xdma("xk", off, w_)
                if first and j == 0:
                    # W for the V projection after the first xk piece
                    nc.sync.dma_start(out=wt[:, 2 * NW:], in_=w_d[:, 2 * NW:])
                    first = False
                xdma("xv", off, w_)
            if LK // 128 <= 8:
                for off, w_ in qp[1:]:
                    xdma("xq", off, w_)

        # ---- per-section compute; later sections' projections fill PE
        # gaps while earlier sections' attention is ACT-bound ----
        for i, d in enumerate(secs):
            LQ, LK, NKC = d["LQ"], d["LK"], d["NKC"]

            qT = qkpool.tile([128, LQ], BF16, tag=f"qT{i}", name=f"qT{i}")
            kT = qkpool.tile([128, LK], BF16, tag=f"kT{i}", name=f"kT{i}")
            v = vpool.tile([128, NKC * 130], BF16, tag=f"v{i}", name=f"v{i}")
            v4 = v.rearrange("p (m h c) -> p m h c", h=2, c=65)
            nc.vector.memset(v4[:, :, :, 64:65], 1.0)

            def proj_tile(proj, xkey, L, dst, off, tw):
                pj = pjp.tile([128, 512], F32, tag="pj", name=f"pj{i}{proj}{off}")
                for kc in range(KC):
                    nc.tensor.matmul(
                        pj[:, 0:tw],
                        lhsT=wt[:, (proj * KC + kc) * HW:(proj * KC + kc + 1) * HW],
                        rhs=d[xkey][:, kc * L + off:kc * L + off + tw],
                        start=(kc == 0), stop=(kc == KC - 1),
                    )
                nc.vector.tensor_copy(dst[:, off:off + tw], pj[:, 0:tw])

            def vproj(mc):
                pv = pjp.tile([128, 128], F32, tag="pj", name=f"pv{i}{mc}")
                for kc in range(KC):
                    nc.tensor.matmul(
                        pv,
                        lhsT=d["xv"][:, kc * LK + mc * 128:kc * LK + (mc + 1) * 128],
                        rhs=wt[:, (2 * KC + kc) * HW:(2 * KC + kc + 1) * HW],
                        start=(kc == 0), stop=(kc == KC - 1),
                    )
                nc.vector.tensor_copy(
                    v4[:, mc:mc + 1, :, 0:64],
                    pv.rearrange("p (o h c) -> p o h c", o=1, c=64))

            # program order mirrors DMA arrival: qT tile0, then (kT tile,
            # its v chunks) per arriving k/v piece, then the remaining qT
            # tiles interleaved between attention qtiles.
            proj_tile(0, "xq", LQ, qT, *d["qtiles"][0])

            # Flash-style key-range passes for long-key sections: both
            # qtiles consume each arriving k/v piece (partial O + partial
            # denominators staged in SBUF, summed at the end) so ScalarE
            # stays fed while the k/v stream is still arriving.
            multi = NKC > 8
            passes = [(0, 10), (10, NKC)] if multi else [(0, NKC)]
            if multi:
                for off2, tw2 in d["qtiles"][1:]:
                    proj_tile(0, "xq", LQ, qT, off2, tw2)
            kvps = _pieces(LK)
            for pi, (kp0, kp1) in enumerate(passes):
                for off, tw in kvps:
                    if kp0 <= off // 128 < kp1:
                        proj_tile(1, "xk", LK, kT, off, tw)
                        for mc in range(off // 128, (off + tw) // 128):
                            vproj(mc)
                for nq, (qt, qw) in enumerate(d["qtiles"]):
                    if not multi and nq + 1 < len(d["qtiles"]):
                        proj_tile(0, "xq", LQ, qT, *d["qtiles"][nq + 1])
                    nqc = qw // 128
                    nA = min(nqc, 3)
                    oA = apA.tile([128, nA * 130], F32, tag="avA",
                                  name=f"oA{i}_{pi}_{nq}")
                    oB = (apB.tile([128, 130], F32, tag="avB",
                                   name=f"oB{i}_{pi}_{nq}")
                          if nqc == 4 else None)

                    def emit_av(e, kc):
                        # PSUM `start` zeroes the whole bank, so each bank
                        # gets exactly one start (its first matmul); later
                        # disjoint slices accumulate onto the cleared bank.
                        for qc in range(nqc):
                            dst = oA if qc < 3 else oB
                            pos = qc % 3
                            for h in range(2):
                                nc.tensor.matmul(
                                    dst[:, pos * 130 + h * 65:
                                        pos * 130 + (h + 1) * 65],
                                    lhsT=e[:, h * 512 + qc * 128:
                                           h * 512 + (qc + 1) * 128],
                                    rhs=v4[:, kc:kc + 1, h:h + 1, :],
                                    start=(kc == kp0 and pos == 0 and h == 0),
                                    stop=(kc == kp1 - 1 and h == 1
                                          and pos == (nA - 1 if qc < 3 else 0)),
                                    skip_group_check=True,
                                )

                    # software pipeline: emit [S + exp for kc][A@V for kc-1]
                    # so the in-order PE stream never waits on ScalarE.
                    prev = None
                    for kc in range(kp0, kp1):
                        s = spool.tile([128, 1024], F32, tag="s",
                                       name=f"s{i}_{pi}_{nq}_{kc}")
                        s3 = s.rearrange("p (h q) -> p h q", q=512)
                        for h in range(2):
                            # each head's slice in its own PSUM bank (two
                            # concurrent tile_position matmuls into one bank
                            # is fatal on HW)
                            nc.tensor.matmul(
                                s3[:, h:h + 1, 0:qw],
                                lhsT=kT[h * 64:(h + 1) * 64,
                                        kc * 128:(kc + 1) * 128],
                                rhs=qT[h * 64:(h + 1) * 64, qt:qt + qw],
                                start=True, stop=True,
                                tile_position=(h * 64, 0),
                            )
                        e = epool.tile([128, 1024], BF16, tag="e",
                                       name=f"e{i}_{pi}_{nq}_{kc}")
                        e3 = e.rearrange("p (h q) -> p h q", q=512)
                        bias = kb[:, i:i + 1] if kc == NKC - 1 else 0.0
                        nc.scalar.activation(e3[:, :, 0:qw], s3[:, :, 0:qw],
                                             EXP, bias=bias, scale=0.125)
                        if prev is not None:
                            emit_av(prev, kc - 1)
                        prev = e
                    emit_av(prev, kp1 - 1)

                    if multi and pi == 0:
                        # stage the first pass's partials in fp32 SBUF
                        stP = stpool.tile([128, nqc * 130], F32,
                                          tag=f"stP{nq}", name=f"stP{i}_{nq}")
                        nc.vector.tensor_copy(stP[:, 0:nA * 130], oA)
                        if oB is not None:
                            nc.vector.tensor_copy(stP[:, 390:520], oB)
                        d[f"stP{nq}"] = stP
                        continue
                    # final pass: combine partials, DMA out as bf16 with
                    # query-chunk PAIRS packed per DRAM row (520B
                    # descriptors).  Host unpacks + divides by denominator.
                    st = stpool.tile([128, nqc * 130], BF16, tag="stA",
                                     name=f"st{i}_{nq}")
                    if multi:
                        stP = d[f"stP{nq}"]
                        nc.vector.tensor_add(st[:, 0:nA * 130], oA,
                                             stP[:, 0:nA * 130])
                        if oB is not None:
                            nc.vector.tensor_add(st[:, 390:520], oB,
                                                 stP[:, 390:520])
                    else:
                        nc.vector.tensor_copy(st[:, 0:nA * 130], oA)
                        if oB is not None:
                            nc.vector.tensor_copy(st[:, 390:520], oB)
                    npair = nqc // 2
                    if npair:
                        P0 = qt // 256
                        nc.sync.dma_start(
                            out=d["out2_d"][P0 * 128:(P0 + npair) * 128, :]
                            .rearrange("(c p) f -> p c f", p=128),
                            in_=st.rearrange("p (c f) -> p c f",
                                             f=260)[:, 0:npair, :])
                    if nqc % 2:
                        nc.sync.dma_start(out=d["out1_d"][:, :],
                                          in_=st[:, (nqc - 1) * 130:nqc * 130])

    nc.compile()
    _nc_cache[cfgs] = nc
    return nc


def _pack_xt(x: np.ndarray, L: int) -> np.ndarray:
    """[L_full, 1024] fp32 -> [128, KC*L] bf16, chunk-major transposed."""
    xt = np.ascontiguousarray(x[:L, :].T)           # [1024, L]
    xt = xt.reshape(KC, 128, L).transpose(1, 0, 2)  # [128, KC, L]
    return np.ascontiguousarray(xt.reshape(128, KC * L)).astype(ml_dtypes.bfloat16)


def _pack_w(W: np.ndarray, g: int) -> np.ndarray:
    """[1024, 1024] -> this core's [128, KC*128] slice, chunk-major."""
    wg = W[:, g * HW:(g + 1) * HW].reshape(KC, 128, HW).transpose(1, 0, 2)
    return np.ascontiguousarray(wg.reshape(128, KC * HW))


def kernel(Q_seq, K_seq, V_seq, q_len, v_len, WQ, WK, WV):
    Q_seq = np.asarray(Q_seq, dtype=np.float32)
    K_seq = np.asarray(K_seq, dtype=np.float32)
    V_seq = np.asarray(V_seq, dtype=np.float32)
    WQ = np.asarray(WQ, dtype=np.float32)
    WK = np.asarray(WK, dtype=np.float32)
    WV = np.asarray(WV, dtype=np.float32)
    ql = np.asarray(q_len).ravel().astype(np.int64)
    vl = np.asarray(v_len).ravel().astype(np.int64)
    B = Q_seq.shape[0]

    cfgs, order = _cfgs_for(ql, vl)
    nc = _build(cfgs)

    kb = np.zeros((128, len(cfgs)), dtype=np.float32)
    shared = {}
    for sec, b in enumerate(order):
        LQ, LK = cfgs[sec]
        shared[f"xq{sec}"] = _pack_xt(Q_seq[b], LQ)
        shared[f"xk{sec}"] = _pack_xt(K_seq[b], LK)
        shared[f"xv{sec}"] = _pack_xt(V_seq[b], LK)
        j0 = LK - 128
        kb[:, sec] = np.where(np.arange(j0, LK) < vl[b], 0.0, -np.float32(NEG_BIG))

    in_maps = []
    for g in range(N_CORES):
        m = dict(shared)
        m["w"] = np.concatenate(
            [_pack_w(WQ, g), _pack_w(WK, g), _pack_w(WV, g)],
            axis=1).astype(ml_dtypes.bfloat16)
        m["kb"] = kb
        in_maps.append(m)

    res = run_bass_kernel_spmd(nc, in_maps, list(range(N_CORES)), trace=TRACE)
    kernel.last_results = [res]
    kernel.last_exec_ns = res.exec_time_ns or 0

    O = np.zeros((B, L_FULL, D_MODEL), dtype=np.float32)
    for sec, b in enumerate(order):
        LQ = cfgs[sec][0]
        n = int(ql[b])
        NP = (LQ // 128) // 2
        for g in range(N_CORES):
            full = np.empty((LQ, 130), dtype=np.float32)
            o2 = np.asarray(res.results[g][f"out2{sec}"], dtype=np.float32)
            full[:NP * 256] = o2.reshape(NP, 128, 2, 130).transpose(
                0, 2, 1, 3).reshape(NP * 256, 130)
            if (LQ // 128) % 2:
                full[NP * 256:] = np.asarray(res.results[g][f"out1{sec}"],
                                             dtype=np.float32)
            o = full.reshape(LQ, 2, 65)
            O[b, :n, g * HW:(g + 1) * HW] = (
                o[:n, :, 0:64] / o[:n, :, 64:65]).reshape(n, HW)
    return O


# revision 26
# speedup vs baseline: 1.0357x; 1.0249x over previous
"""Multi-head self-attention (B=2, L=2048, H=16, dh=64) on 8 TRN2 NeuronCores.

Strategy (v8):
  - One SPMD launch; each core runs one head-pair (2 heads) of EVERY batch,
    as straight-line sections with per-batch loop bounds (padded to 128).
  - All X/W shipped bf16, host-packed [128, KC*L] chunk-major; DMAed in
    512-column pieces ordered so compute can chase the DMA stream
    (the input stream, not compute, bounds the start of the big batch):
    per section [xq piece0, (xk,xv) piece pairs, remaining xq pieces].
  - A short warm-up matmul chain on memset data ramps the PE p-state to
    full clock before real work arrives.
  - qT/kT = W.T @ X.T projections -> PSUM -> bf16 SBUF; V projected in
    [key, dh] orientation; k-proj and v-proj interleaved per arriving
    DMA piece (the engines execute their static streams IN ORDER, so a
    stalled producer ahead in the stream blocks everything behind it).
  - S^T[k, q] per head via paired K=64 matmuls (tile_position packing),
    each head's 512-wide slice in its own PSUM bank.
  - exp on ScalarE straight from PSUM; 1/sqrt(dh) folded into the
    activation scale; additive key mask only for the final (partial) key
    chunk; bf16 output.
  - A@V with the exp tile stationary and ones-augmented V moving (N=65):
    accumulates directly as [query, head*65+d] with the softmax
    denominator in column 64 -- no transposes.  PSUM `start` zeroes a
    whole bank, so each accumulator bank gets exactly one start.  The
    A@V for key chunk kc is emitted after S/exp of kc+1 (software
    pipelining).  Host performs the final divide and query-length crop.
    Output DMAs ride the GpSimd SWDGE queue, as bf16.
"""

import math
from contextlib import ExitStack

import ml_dtypes
import numpy as np

import concourse.mybir as mybir
import concourse.tile as tile
from concourse import bacc
from concourse.bass_utils import run_bass_kernel_spmd

F32 = mybir.dt.float32
BF16 = mybir.dt.bfloat16
EXP = mybir.ActivationFunctionType.Exp
NEG_BIG = 1e12

D_MODEL = 1024
L_FULL = 2048
DH = 64
N_CORES = 8
KC = D_MODEL // 128    # contraction chunks
HW = 128               # one head-pair (2 heads) per core

_nc_cache: dict = {}
TRACE = False


def _pad128(n: int) -> int:
    return min(L_FULL, max(128, int(math.ceil(n / 128)) * 128))


def _cfgs_for(ql, vl):
    """Section configs, smallest DMA footprint first."""
    B = len(ql)
    order = sorted(range(B), key=lambda b: _pad128(int(ql[b])) + 2 * _pad128(int(vl[b])))
    return tuple((_pad128(int(ql[b])), _pad128(int(vl[b]))) for b in order), order


def _pieces(L, w=512):
    """w-wide column pieces (merge a <256 tail so DMA elems stay >=512B)."""
    ps = [(o, min(w, L - o)) for o in range(0, L, w)]
    if len(ps) > 1 and ps[-1][1] < 256:
        (o, pw), (_, wt) = ps[-2], ps[-1]
        ps[-2:] = [(o, pw + wt)]
    return ps


def _build(cfgs: tuple):
    """cfgs: tuple of (LQ, LK) per batch section."""
    if cfgs in _nc_cache:
        return _nc_cache[cfgs]

    nc = bacc.Bacc("TRN2", target_bir_lowering=False, debug=False,
                   num_devices=N_CORES)
    nsec = len(cfgs)

    w_d = nc.dram_tensor("w", [128, 3 * KC * HW], BF16, kind="ExternalInput")
    kb_d = nc.dram_tensor("kb", [128, nsec], F32, kind="ExternalInput")
    secs = []
    for i, (LQ, LK) in enumerate(cfgs):
        d = dict(LQ=LQ, LK=LK, NKC=LK // 128)
        d["qtiles"] = [(o, min(512, LQ - o)) for o in range(0, LQ, 512)]
        d["xq_d"] = nc.dram_tensor(f"xq{i}", [128, KC * LQ], BF16, kind="ExternalInput")
        d["xk_d"] = nc.dram_tensor(f"xk{i}", [128, KC * LK], BF16, kind="ExternalInput")
        d["xv_d"] = nc.dram_tensor(f"xv{i}", [128, KC * LK], BF16, kind="ExternalInput")
        NP = (LQ // 128) // 2
        d["NP"] = NP
        d["out2_d"] = nc.dram_tensor(f"out2{i}", [NP * 128, 260], BF16,
                                     kind="ExternalOutput")
        if (LQ // 128) % 2:
            d["out1_d"] = nc.dram_tensor(f"out1{i}", [128, 130], BF16,
                                         kind="ExternalOutput")
        secs.append(d)

    with ExitStack() as ctx:
        tc = ctx.enter_context(tile.TileContext(nc))
        const = ctx.enter_context(tc.tile_pool(name="const", bufs=1))
        xpool = ctx.enter_context(tc.tile_pool(name="xp", bufs=1))
        qkpool = ctx.enter_context(tc.tile_pool(name="qk", bufs=1))
        vpool = ctx.enter_context(tc.tile_pool(name="vp", bufs=1))
        epool = ctx.enter_context(tc.tile_pool(name="ep", bufs=6))
        stpool = ctx.enter_context(tc.tile_pool(name="st", bufs=4))
        # PSUM (8 banks): 2 x 2-bank score tiles + 1+1 banks of O
        # accumulators + 2 x 1-bank projection slots.
        spool = ctx.enter_context(tc.tile_pool(name="ps_s", bufs=2, space="PSUM"))
        apA = ctx.enter_context(tc.tile_pool(name="ps_a", bufs=1, space="PSUM"))
        apB = ctx.enter_context(tc.tile_pool(name="ps_b", bufs=1, space="PSUM"))
        pjp = ctx.enter_context(tc.tile_pool(name="ps_pj", bufs=2, space="PSUM"))

        # ---- PE p-state warm-up: ~3.5us of throwaway matmuls on memset
        # data so the real stream starts at full clock.
        wu = const.tile([128, 512], BF16, name="wu")
        nc.vector.memset(wu, 0.0)
        wup = apA.tile([128, 390], F32, tag="avA", name="wup")
        for r in range(8):
            nc.tensor.matmul(wup, lhsT=wu[:, 0:128], rhs=wu[:, 0:390],
                             start=True, stop=True)

        # ---- input DMAs, arrival order = consumption order ----
        wt = const.tile([128, 3 * KC * HW], BF16, name="w")
        nc.sync.dma_start(out=wt[:, 0:KC * HW], in_=w_d[:, 0:KC * HW])
        kb = const.tile([128, nsec], F32, name="kb")
        nc.sync.dma_start(out=kb, in_=kb_d[:, :])
        NW = KC * HW
        first = True
        for i, d in enumerate(secs):
            LQ, LK = d["LQ"], d["LK"]
            for key, L in (("xq", LQ), ("xk", LK), ("xv", LK)):
                t = xpool.tile([128, KC * L], BF16, tag=f"{key}{i}", name=f"{key}{i}")
                d[key] = t
                d[key + "3"] = t.rearrange("p (k l) -> p k l", l=L)
                d[key + "_d3"] = d[key + "_d"][:, :].rearrange("p (k l) -> p k l", l=L)

            def xdma(key, off, w_):
                nc.sync.dma_start(out=d[key + "3"][:, :, off:off + w_],
                                  in_=d[key + "_d3"][:, :, off:off + w_])

            qp = _pieces(LQ)
            xdma("xq", *qp[0])
            if first:
                # W for the K projection rides after the first xq piece
                nc.sync.dma_start(out=wt[:, NW:2 * NW], in_=w_d[:, NW:2 * NW])
            if LK // 128 > 8:
                # multi-pass section: both qtiles run per pass, so all qT
                # pieces must precede the k/v stream
                for off, w_ in qp[1:]:
                    xdma("xq", off, w_)
            for j, (off, w_) in enumerate(_pieces(LK)):
                xdma("xk", off, w_)
                if first and j == 0:
                    # W for the V projection after the first xk piece
                    nc.sync.dma_start(out=wt[:, 2 * NW:], in_=w_d[:, 2 * NW:])
                    first = False
                xdma("xv", off, w_)
            if LK // 128 <= 8:
                for off, w_ in qp[1:]:
                    xdma("xq", off, w_)

        # ---- per-section compute; later sections' projections fill PE
        # gaps while earlier sections' attention is ACT-bound ----
        for i, d in enumerate(secs):
            LQ, LK, NKC = d["LQ"], d["LK"], d["NKC"]

            qT = qkpool.tile([128, LQ], BF16, tag=f"qT{i}", name=f"qT{i}")
            kT = qkpool.tile([128, LK], BF16, tag=f"kT{i}", name=f"kT{i}")
            v = vpool.tile([128, NKC * 130], BF16, tag=f"v{i}", name=f"v{i}")
            v4 = v.rearrange("p (m h c) -> p m h c", h=2, c=65)
            nc.vector.memset(v4[:, :, :, 64:65], 1.0)

            def proj_tile(proj, xkey, L, dst, off, tw):
                pj = pjp.tile([128, 512], F32, tag="pj", name=f"pj{i}{proj}{off}")
                for kc in range(KC):
                    nc.tensor.matmul(
                        pj[:, 0:tw],
                        lhsT=wt[:, (proj * KC + kc) * HW:(proj * KC + kc + 1) * HW],
                        rhs=d[xkey][:, kc * L + off:kc * L + off + tw],
                        start=(kc == 0), stop=(kc == KC - 1),
                    )
                nc.vector.tensor_copy(dst[:, off:off + tw], pj[:, 0:tw])

            def vproj(mc):
                pv = pjp.tile([128, 128], F32, tag="pj", name=f"pv{i}{mc}")
                for kc in range(KC):
                    nc.tensor.matmul(
                        pv,
                        lhsT=d["xv"][:, kc * LK + mc * 128:kc * LK + (mc + 1) * 128],
                        rhs=wt[:, (2 * KC + kc) * HW:(2 * KC + kc + 1) * HW],
                        start=(kc == 0), stop=(kc == KC - 1),
                    )
                nc.vector.tensor_copy(
                    v4[:, mc:mc + 1, :, 0:64],
                    pv.rearrange("p (o h c) -> p o h c", o=1, c=64))

            # program order mirrors DMA arrival: qT tile0, then (kT tile,
            # its v chunks) per arriving k/v piece, then the remaining qT
            # tiles interleaved between attention qtiles.
            proj_tile(0, "xq", LQ, qT, *d["qtiles"][0])

            # Flash-style key-range passes for long-key sections: both
            # qtiles consume each arriving k/v piece (partial O + partial
            # denominators staged in SBUF, summed at the end) so ScalarE
            # stays fed while the k/v stream is still arriving.
            multi = NKC > 8
            passes = [(0, 8), (8, NKC)] if multi else [(0, NKC)]
            if multi:
                for off2, tw2 in d["qtiles"][1:]:
                    proj_tile(0, "xq", LQ, qT, off2, tw2)
            kvps = _pieces(LK)
            for pi, (kp0, kp1) in enumerate(passes):
                for off, tw in kvps:
                    if kp0 <= off // 128 < kp1:
                        # 256-wide kT tiles: halves the S-chain's wait on
                        # each arriving k piece at negligible extra rows
                        for o2 in range(off, off + tw, 256):
                            proj_tile(1, "xk", LK, kT, o2, min(256, off + tw - o2))
                        for mc in range(off // 128, (off + tw) // 128):
                            vproj(mc)
                for nq, (qt, qw) in enumerate(d["qtiles"]):
                    if not multi and nq + 1 < len(d["qtiles"]):
                        proj_tile(0, "xq", LQ, qT, *d["qtiles"][nq + 1])
                    nqc = qw // 128
                    nA = min(nqc, 3)
                    oA = apA.tile([128, nA * 130], F32, tag="avA",
                                  name=f"oA{i}_{pi}_{nq}")
                    oB = (apB.tile([128, 130], F32, tag="avB",
                                   name=f"oB{i}_{pi}_{nq}")
                          if nqc == 4 else None)

                    def emit_av(e, kc):
                        # PSUM `start` zeroes the whole bank, so each bank
                        # gets exactly one start (its first matmul); later
                        # disjoint slices accumulate onto the cleared bank.
                        for qc in range(nqc):
                            dst = oA if qc < 3 else oB
                            pos = qc % 3
                            for h in range(2):
                                nc.tensor.matmul(
                                    dst[:, pos * 130 + h * 65:
                                        pos * 130 + (h + 1) * 65],
                                    lhsT=e[:, h * 512 + qc * 128:
                                           h * 512 + (qc + 1) * 128],
                                    rhs=v4[:, kc:kc + 1, h:h + 1, :],
                                    start=(kc == kp0 and pos == 0 and h == 0),
                                    stop=(kc == kp1 - 1 and h == 1
                                          and pos == (nA - 1 if qc < 3 else 0)),
                                    skip_group_check=True,
                                )

                    # software pipeline: emit [S + exp for kc][A@V for kc-1]
                    # so the in-order PE stream never waits on ScalarE.
                    prev = None
                    for kc in range(kp0, kp1):
                        s = spool.tile([128, 1024], F32, tag="s",
                                       name=f"s{i}_{pi}_{nq}_{kc}")
                        s3 = s.rearrange("p (h q) -> p h q", q=512)
                        for h in range(2):
                            # each head's slice in its own PSUM bank (two
                            # concurrent tile_position matmuls into one bank
                            # is fatal on HW)
                            nc.tensor.matmul(
                                s3[:, h:h + 1, 0:qw],
                                lhsT=kT[h * 64:(h + 1) * 64,
                                        kc * 128:(kc + 1) * 128],
                                rhs=qT[h * 64:(h + 1) * 64, qt:qt + qw],
                                start=True, stop=True,
                                tile_position=(h * 64, 0),
                            )
                        e = epool.tile([128, 1024], BF16, tag="e",
                                       name=f"e{i}_{pi}_{nq}_{kc}")
                        e3 = e.rearrange("p (h q) -> p h q", q=512)
                        bias = kb[:, i:i + 1] if kc == NKC - 1 else 0.0
                        nc.scalar.activation(e3[:, :, 0:qw], s3[:, :, 0:qw],
                                             EXP, bias=bias, scale=0.125)
                        if prev is not None:
                            emit_av(prev, kc - 1)
                        prev = e
                    emit_av(prev, kp1 - 1)

                    if multi and pi == 0:
                        # stage the first pass's partials in fp32 SBUF
                        stP = stpool.tile([128, nqc * 130], F32,
                                          tag=f"stP{nq}", name=f"stP{i}_{nq}")
                        nc.vector.tensor_copy(stP[:, 0:nA * 130], oA)
                        if oB is not None:
                            nc.vector.tensor_copy(stP[:, 390:520], oB)
                        d[f"stP{nq}"] = stP
                        continue
                    # final pass: combine partials, DMA out as bf16 with
                    # query-chunk PAIRS packed per DRAM row (520B
                    # descriptors).  Host unpacks + divides by denominator.
                    st = stpool.tile([128, nqc * 130], BF16, tag="stA",
                                     name=f"st{i}_{nq}")
                    if multi:
                        stP = d[f"stP{nq}"]
                        nc.vector.tensor_add(st[:, 0:nA * 130], oA,
                                             stP[:, 0:nA * 130])
                        if oB is not None:
                            nc.vector.tensor_add(st[:, 390:520], oB,
                                                 stP[:, 390:520])
                    else:
                        nc.vector.tensor_copy(st[:, 0:nA * 130], oA)
                        if oB is not None:
                            nc.vector.tensor_copy(st[:, 390:520], oB)
                    npair = nqc // 2
                    if npair:
                        P0 = qt // 256
                        nc.sync.dma_start(
                            out=d["out2_d"][P0 * 128:(P0 + npair) * 128, :]
                            .rearrange("(c p) f -> p c f", p=128),
                            in_=st.rearrange("p (c f) -> p c f",
                                             f=260)[:, 0:npair, :])
                    if nqc % 2:
                        nc.sync.dma_start(out=d["out1_d"][:, :],
                                          in_=st[:, (nqc - 1) * 130:nqc * 130])

    nc.compile()
    _nc_cache[cfgs] = nc
    return nc


def _pack_xt(x: np.ndarray, L: int) -> np.ndarray:
    """[L_full, 1024] fp32 -> [128, KC*L] bf16, chunk-major transposed."""
    xt = np.ascontiguousarray(x[:L, :].T)           # [1024, L]
    xt = xt.reshape(KC, 128, L).transpose(1, 0, 2)  # [128, KC, L]
    return np.ascontiguousarray(xt.reshape(128, KC * L)).astype(ml_dtypes.bfloat16)


def _pack_w(W: np.ndarray, g: int) -> np.ndarray:
    """[1024, 1024] -> this core's [128, KC*128] slice, chunk-major."""
    wg = W[:, g * HW:(g + 1) * HW].reshape(KC, 128, HW).transpose(1, 0, 2)
    return np.ascontiguousarray(wg.reshape(128, KC * HW))


def kernel(Q_seq, K_seq, V_seq, q_len, v_len, WQ, WK, WV):
    Q_seq = np.asarray(Q_seq, dtype=np.float32)
    K_seq = np.asarray(K_seq, dtype=np.float32)
    V_seq = np.asarray(V_seq, dtype=np.float32)
    WQ = np.asarray(WQ, dtype=np.float32)
    WK = np.asarray(WK, dtype=np.float32)
    WV = np.asarray(WV, dtype=np.float32)
    ql = np.asarray(q_len).ravel().astype(np.int64)
    vl = np.asarray(v_len).ravel().astype(np.int64)
    B = Q_seq.shape[0]

    cfgs, order = _cfgs_for(ql, vl)
    nc = _build(cfgs)

    kb = np.zeros((128, len(cfgs)), dtype=np.float32)
    shared = {}
    for sec, b in enumerate(order):
        LQ, LK = cfgs[sec]
        shared[f"xq{sec}"] = _pack_xt(Q_seq[b], LQ)
        shared[f"xk{sec}"] = _pack_xt(K_seq[b], LK)
        shared[f"xv{sec}"] = _pack_xt(V_seq[b], LK)
        j0 = LK - 128
        kb[:, sec] = np.where(np.arange(j0, LK) < vl[b], 0.0, -np.float32(NEG_BIG))

    in_maps = []
    for g in range(N_CORES):
        m = dict(shared)
        m["w"] = np.concatenate(
            [_pack_w(WQ, g), _pack_w(WK, g), _pack_w(WV, g)],
            axis=1).astype(ml_dtypes.bfloat16)
        m["kb"] = kb
        in_maps.append(m)

    res = run_bass_kernel_spmd(nc, in_maps, list(range(N_CORES)), trace=TRACE)
    kernel.last_results = [res]
    kernel.last_exec_ns = res.exec_time_ns or 0

    O = np.zeros((B, L_FULL, D_MODEL), dtype=np.float32)
    for sec, b in enumerate(order):
        LQ = cfgs[sec][0]
        n = int(ql[b])
        NP = (LQ // 128) // 2
        for g in range(N_CORES):
            full = np.empty((LQ, 130), dtype=np.float32)
            o2 = np.asarray(res.results[g][f"out2{sec}"], dtype=np.float32)
            full[:NP * 256] = o2.reshape(NP, 128, 2, 130).transpose(
                0, 2, 1, 3).reshape(NP * 256, 130)
            if (LQ // 128) % 2:
                full[NP * 256:] = np.asarray(res.results[g][f"out1{sec}"],
                                             dtype=np.float32)
            o = full.reshape(LQ, 2, 65)
            O[b, :n, g * HW:(g + 1) * HW] = (
                o[:n, :, 0:64] / o[:n, :, 64:65]).reshape(n, HW)
    return O
